# revision 4
# baseline (speedup 1.0000x reference)
"""Trainium2 Bass kernel for multi-head location-aware attention.

Full-input contract: kernel(**inputs) takes the unsharded numpy inputs and
returns (c, w) like the reference. Internally shards batch B=32 across 8
NeuronCores (4 batches/core, all 4 heads per core).

Math notes:
  - v-projection is algebraically eliminated: c = ((w @ enc) @ Wv) @ Wo.
  - gb cancels in softmax (shift invariance) and is dropped.
  - q-projection bias bq is folded via an augmented row (ones) in dec_z^T.
  - masking is host-precomputed as additive -1e30 rows (bf16).
"""

import numpy as np
import ml_dtypes
from contextlib import ExitStack

import concourse.bass as bass
import concourse.tile as tile
from concourse import bacc, mybir
from concourse.bass_utils import run_bass_kernel_spmd

FP32 = mybir.dt.float32
FP32R = mybir.dt.float32r
BF16 = mybir.dt.bfloat16
AF = mybir.ActivationFunctionType
AX = mybir.AxisListType
OP = mybir.AluOpType

# problem dims (hardcoded)
B, T, E, D, H, DK, DV, C, FILT = 32, 2048, 1024, 1024, 4, 512, 512, 10, 100
KW = 2 * FILT + 1            # 201
NCORES = 8
BL = B // NCORES             # 4 batches per core
TP = T + 2 * FILT            # 2248 padded att length
SCALING = 2.0
DQ = 1152                    # 1024 + 1 (ones row) padded to 9*128

NTT = T // 128               # 16 t-tiles
NEE = E // 128               # 8 e-tiles
NMK = DK // 128              # 4 dk-tiles
NCH = T // 512               # 4 t-chunks of 512
NKQ = DQ // 128              # 9 q contraction tiles

_CACHED_NC = None


def build_nc():
    nc = bacc.Bacc("TRN2", target_bir_lowering=False, debug=False)

    enc = nc.dram_tensor("enc", [BL, T, E], FP32, kind="ExternalInput").ap()
    att_pad = nc.dram_tensor("att_pad", [BL, H, TP], FP32R, kind="ExternalInput").ap()
    masknb = nc.dram_tensor("masknb", [BL, H, T], BF16, kind="ExternalInput").ap()
    qz = nc.dram_tensor("qz", [DQ, BL], FP32R, kind="ExternalInput").ap()
    wq = nc.dram_tensor("wq", [H, DQ, DK], FP32R, kind="ExternalInput").ap()
    wk = nc.dram_tensor("wk", [H, E, DK], FP32R, kind="ExternalInput").ap()
    watt = nc.dram_tensor("watt", [H, C, DK], FP32R, kind="ExternalInput").ap()
    cwt = nc.dram_tensor("cwt", [H, KW, C], FP32R, kind="ExternalInput").ap()
    gw = nc.dram_tensor("gw", [H, DK], FP32R, kind="ExternalInput").ap()
    wv = nc.dram_tensor("wv", [H, E, DV], FP32R, kind="ExternalInput").ap()
    wo = nc.dram_tensor("wo", [H * DV, E], FP32R, kind="ExternalInput").ap()
    ident = nc.dram_tensor("ident", [128, 128], FP32, kind="ExternalInput").ap()

    c_out = nc.dram_tensor("c_out", [BL, E], FP32, kind="ExternalOutput").ap()
    w_out = nc.dram_tensor("w_out", [BL, H, T], FP32, kind="ExternalOutput").ap()

    with tile.TileContext(nc) as tc, ExitStack() as ctx:
        # ---------------- persistent pools ----------------
        const_p = ctx.enter_context(tc.tile_pool(name="const", bufs=1))
        wk_p = ctx.enter_context(tc.tile_pool(name="wkp", bufs=1))
        persist_p = ctx.enter_context(tc.tile_pool(name="persistp", bufs=1))
        sm_p = ctx.enter_context(tc.tile_pool(name="smp", bufs=1))

        # PSUM pools (8 banks total: 2+2+1+1+2)
        ps_mm = ctx.enter_context(tc.tile_pool(name="psmm", bufs=2, space="PSUM"))
        ps_tp = ctx.enter_context(tc.tile_pool(name="pstp", bufs=2, space="PSUM"))
        ps_e = ctx.enter_context(tc.tile_pool(name="pse", bufs=1, space="PSUM"))
        ps_cv = ctx.enter_context(tc.tile_pool(name="pscv", bufs=1, space="PSUM"))
        ps_misc = ctx.enter_context(tc.tile_pool(name="psmisc", bufs=1, space="PSUM"))

        # ---------------- constants / weights ----------------
        id_sb = const_p.tile([128, 128], FP32, tag="id")
        nc.sync.dma_start(id_sb[:], ident)

        wk_sb = wk_p.tile([128, H, NEE, DK], FP32R, tag="wk")
        for h in range(H):
            for ke in range(NEE):
                nc.sync.dma_start(
                    wk_sb[:, h, ke, :], wk[h, ke * 128:(ke + 1) * 128, :]
                )

        watt_sb = const_p.tile([C, H, DK], FP32R, tag="watt")
        cw0_sb = const_p.tile([128, H, C], FP32R, tag="cw0")
        cw1_sb = const_p.tile([KW - 128, H, C], FP32R, tag="cw1")
        gw_sb = const_p.tile([128, H, NMK], FP32R, tag="gw")
        for h in range(H):
            nc.sync.dma_start(watt_sb[:, h, :], watt[h])
            nc.sync.dma_start(cw0_sb[:, h, :], cwt[h, 0:128, :])
            nc.sync.dma_start(cw1_sb[:, h, :], cwt[h, 128:KW, :])
            nc.sync.dma_start(
                gw_sb[:, h, :], bass.AP(gw.tensor, h * DK, [[1, 128], [128, NMK]])
            )

        mask_sb = const_p.tile([H, BL, T], BF16, tag="mask")
        for b in range(BL):
            nc.sync.dma_start(mask_sb[:, b, :], masknb[b])

        # persistent cross-phase tiles
        q_sb = persist_p.tile([128, H, NMK, BL], FP32, tag="q")
        ctxT_sb = persist_p.tile([128, NEE, BL, H], FP32R, tag="ctxT")
        wT_sb = persist_p.tile([128, BL, NTT, H], BF16, tag="wT")

        # ---------------- q projection (once) ----------------
        with tc.tile_pool(name="wqp", bufs=2) as wq_p:
            qz_sb = const_p.tile([128, NKQ, BL], FP32R, tag="qz")
            for kq in range(NKQ):
                nc.sync.dma_start(qz_sb[:, kq, :], qz[kq * 128:(kq + 1) * 128, :])
            for h in range(H):
                for mk in range(NMK):
                    wq_t = wq_p.tile([128, NKQ, 128], FP32R, tag="wqt")
                    for kq in range(NKQ):
                        nc.sync.dma_start(
                            wq_t[:, kq, :],
                            wq[h, kq * 128:(kq + 1) * 128,
                               mk * 128:(mk + 1) * 128],
                        )
                    qps = ps_misc.tile([128, BL], FP32, tag="misc")
                    for kq in range(NKQ):
                        nc.tensor.matmul(
                            qps[:], wq_t[:, kq, :], qz_sb[:, kq, :],
                            start=(kq == 0), stop=(kq == NKQ - 1),
                        )
                    nc.vector.tensor_copy(q_sb[:, h, mk, :], qps[:])

        # ---------------- main loop over local batches ----------------
        with tc.tile_pool(name="encp", bufs=3) as enc_p, \
             tc.tile_pool(name="encTp", bufs=2) as encT_p, \
             tc.tile_pool(name="imp", bufs=2) as im_p, \
             tc.tile_pool(name="convp", bufs=2) as conv_p, \
             tc.tile_pool(name="tanhp", bufs=5) as tanh_p, \
             tc.tile_pool(name="ebp", bufs=4) as eb_p:
            for b in range(BL):
                E_b = sm_p.tile([H, T], FP32, tag="Eb")
                for tch in range(NCH):
                    t0 = tch * 512
                    # --- A: transpose 4 enc t-tiles into encT chunk ---
                    encT = encT_p.tile([128, NEE, 512], FP32R, tag="encT")
                    for ti in range(4):
                        tt = tch * 4 + ti
                        et = enc_p.tile([128, E], FP32, tag="encn")
                        nc.sync.dma_start(et[:], enc[b, tt * 128:(tt + 1) * 128, :])
                        for eg in range(2):
                            tps = ps_tp.tile([128, 512], FP32, tag="tp")
                            for j in range(4):
                                ee = eg * 4 + j
                                nc.tensor.transpose(
                                    tps[:, j * 128:(j + 1) * 128],
                                    et[:, ee * 128:(ee + 1) * 128],
                                    id_sb[:],
                                )
                            nc.vector.tensor_copy(
                                encT[:, eg * 4:(eg + 1) * 4,
                                     ti * 128:(ti + 1) * 128],
                                tps[:].rearrange("p (e t) -> p e t", e=4),
                            )

                    # --- B: scores for this chunk, all heads ---
                    for h in range(H):
                        # conv: (C, 512)
                        im0 = im_p.tile([128, 512], FP32R, tag="im0")
                        im1 = im_p.tile([KW - 128, 512], FP32R, tag="im1")
                        base = (b * H + h) * TP
                        nc.sync.dma_start(
                            im0[:],
                            bass.AP(att_pad.tensor, base + t0,
                                    [[1, 128], [1, 512]]),
                        )
                        nc.sync.dma_start(
                            im1[:],
                            bass.AP(att_pad.tensor, base + t0 + 128,
                                    [[1, KW - 128], [1, 512]]),
                        )
                        cps = ps_cv.tile([C, 512], FP32, tag="cv")
                        nc.tensor.matmul(cps[:], cw0_sb[:, h, :], im0[:],
                                         start=True, stop=False)
                        nc.tensor.matmul(cps[:], cw1_sb[:, h, :], im1[:],
                                         start=False, stop=True)
                        conv_sb = conv_p.tile([C, 512], FP32R, tag="conv")
                        nc.vector.tensor_copy(conv_sb[:], cps[:])

                        # k-proj + loc per dk tile, tanh
                        tanh_tiles = []
                        for mk in range(NMK):
                            kps = ps_mm.tile([128, 512], FP32, tag="kps")
                            for ke in range(NEE):
                                nc.tensor.matmul(
                                    kps[:],
                                    wk_sb[:, h, ke, mk * 128:(mk + 1) * 128],
                                    encT[:, ke, :],
                                    start=(ke == 0), stop=False,
                                )
                            nc.tensor.matmul(
                                kps[:],
                                watt_sb[:, h, mk * 128:(mk + 1) * 128],
                                conv_sb[:],
                                start=False, stop=True,
                            )
                            th = tanh_p.tile([128, 512], FP32R, tag="tanh")
                            nc.scalar.activation(
                                th[:], kps[:], AF.Tanh,
                                bias=q_sb[:, h, mk, b:b + 1], scale=1.0,
                            )
                            tanh_tiles.append(th)

                        # gw dot -> e chunk (1, 512) -> E_b row h
                        eps = ps_e.tile([1, 512], FP32, tag="eps")
                        for mk in range(NMK):
                            nc.tensor.matmul(
                                eps[:], gw_sb[:, h, mk:mk + 1],
                                tanh_tiles[mk][:],
                                start=(mk == 0), stop=(mk == NMK - 1),
                            )
                        esb = eb_p.tile([1, 512], FP32, tag="esb")
                        nc.scalar.copy(esb[:], eps[:])
                        nc.sync.dma_start(E_b[h:h + 1, t0:t0 + 512], esb[:])

                # --- C: softmax over T for 4 heads (in place on E_b) ---
                nc.vector.tensor_tensor(E_b[:], E_b[:], mask_sb[:, b, :], op=OP.add)
                mx = sm_p.tile([H, 1], FP32, tag="mx")
                nc.vector.tensor_reduce(mx[:], E_b[:], AX.X, OP.max)
                m2 = sm_p.tile([H, 1], FP32, tag="m2")
                nc.vector.tensor_scalar_mul(m2[:], mx[:], -SCALING)
                S_b = sm_p.tile([H, 1], FP32, tag="Sb")
                nc.scalar.activation(E_b[:], E_b[:], AF.Exp, bias=m2[:],
                                     scale=SCALING, accum_out=S_b[:])
                sinv = sm_p.tile([H, 1], FP32, tag="sinv")
                nc.vector.reciprocal(sinv[:], S_b[:])
                nc.vector.tensor_scalar_mul(E_b[:], E_b[:], sinv[:])
                nc.sync.dma_start(w_out[b], E_b[:])

                # --- wT: transpose W rows to (t, h) bf16 ---
                for g in range(NTT // 4):
                    tps = ps_tp.tile([128, 512], FP32, tag="tp")
                    for j in range(4):
                        tt = g * 4 + j
                        nc.tensor.transpose(
                            tps[:, j * 128:j * 128 + H],
                            E_b[:, tt * 128:(tt + 1) * 128],
                            id_sb[0:H, 0:H],
                        )
                    for j in range(4):
                        tt = g * 4 + j
                        nc.vector.tensor_copy(
                            wT_sb[:, b, tt, :], tps[:, j * 128:j * 128 + H]
                        )

                # --- D: ctx = w @ enc (bf16) -> transpose -> ctxT (fp32r) ---
                ctxps = ps_misc.tile([H, E], FP32, tag="misc")
                for tt in range(NTT):
                    ebf = enc_p.tile([128, E], BF16, tag="encbf", bufs=2)
                    nc.gpsimd.dma_start(ebf[:], enc[b, tt * 128:(tt + 1) * 128, :])
                    for nch in range(2):
                        nc.tensor.matmul(
                            ctxps[:, nch * 512:(nch + 1) * 512],
                            wT_sb[:, b, tt, :],
                            ebf[:, nch * 512:(nch + 1) * 512],
                            start=(tt == 0), stop=(tt == NTT - 1),
                        )
                ctxN = sm_p.tile([H, E], FP32, tag="ctxN")
                nc.vector.tensor_copy(ctxN[:], ctxps[:])
                for eg in range(2):
                    tps = ps_tp.tile([128, 512], FP32, tag="tp")
                    for j in range(4):
                        ee = eg * 4 + j
                        nc.tensor.transpose(
                            tps[:, j * 128:j * 128 + H],
                            ctxN[:, ee * 128:(ee + 1) * 128],
                            id_sb[0:H, 0:H],
                        )
                    for j in range(4):
                        ee = eg * 4 + j
                        nc.vector.tensor_copy(
                            ctxT_sb[:, ee, b, :], tps[:, j * 128:j * 128 + H]
                        )

        # ---------------- final: ctxv = ctxT.T @ Wv ; c = ctxv.T @ Wo ----
        with tc.tile_pool(name="wvp", bufs=2) as wv_p, \
             tc.tile_pool(name="wop", bufs=2) as wo_p:
            cvT_sb = persist_p.tile([128, H, NMK, BL], FP32R, tag="cvT")
            for h in range(H):
                cvps = ps_cv.tile([BL, DV], FP32, tag="cv")
                for ee in range(NEE):
                    wv_t = wv_p.tile([128, DV], FP32R, tag="wvt")
                    nc.sync.dma_start(
                        wv_t[:], wv[h, ee * 128:(ee + 1) * 128, :]
                    )
                    nc.tensor.matmul(
                        cvps[:], ctxT_sb[:, ee, :, h], wv_t[:],
                        start=(ee == 0), stop=(ee == NEE - 1),
                    )
                cv_sb = sm_p.tile([BL, DV], FP32, tag="cv")
                nc.vector.tensor_copy(cv_sb[:], cvps[:])
                for mv in range(NMK):
                    tps = ps_tp.tile([128, 512], FP32, tag="tp")
                    nc.tensor.transpose(
                        tps[:, 0:BL], cv_sb[:, mv * 128:(mv + 1) * 128],
                        id_sb[0:BL, 0:BL],
                    )
                    nc.vector.tensor_copy(cvT_sb[:, h, mv, :], tps[:, 0:BL])

            cps_f = ps_misc.tile([BL, E], FP32, tag="misc")
            nkt = H * NMK
            k = 0
            for h in range(H):
                for mv in range(NMK):
                    wo_t = wo_p.tile([128, E], FP32R, tag="wot")
                    nc.sync.dma_start(
                        wo_t[:],
                        wo[(h * DV + mv * 128):(h * DV + (mv + 1) * 128), :],
                    )
                    for nch in range(2):
                        nc.tensor.matmul(
                            cps_f[:, nch * 512:(nch + 1) * 512],
                            cvT_sb[:, h, mv, :],
                            wo_t[:, nch * 512:(nch + 1) * 512],
                            start=(k == 0), stop=(k == nkt - 1),
                        )
                    k += 1
            c_sb = sm_p.tile([BL, E], FP32, tag="csb")
            nc.vector.tensor_copy(c_sb[:], cps_f[:])
            nc.sync.dma_start(c_out, c_sb[:])

    nc.compile()
    return nc


def get_nc():
    global _CACHED_NC
    if _CACHED_NC is None:
        _CACHED_NC = build_nc()
    return _CACHED_NC


def make_in_maps(enc_hs_pad, enc_hs_len, dec_z, att_prev, Wq, bq, Wk, Wv, gw,
                 gb, conv_w, Watt, Wo):
    enc_hs_pad = np.ascontiguousarray(enc_hs_pad, dtype=np.float32)
    dec_z = np.asarray(dec_z, dtype=np.float32)
    att_prev = np.asarray(att_prev, dtype=np.float32)
    lens = np.asarray(enc_hs_len)

    wq_aug = np.zeros((H, DQ, DK), dtype=np.float32)
    wq_aug[:, :D, :] = Wq
    wq_aug[:, D, :] = bq
    cwt = np.ascontiguousarray(np.transpose(conv_w, (0, 2, 1)))  # (H, KW, C)
    ident = np.eye(128, dtype=np.float32)
    wk_a = np.ascontiguousarray(Wk, dtype=np.float32)
    wv_a = np.ascontiguousarray(Wv, dtype=np.float32)
    wo_a = np.ascontiguousarray(Wo, dtype=np.float32)
    watt_a = np.ascontiguousarray(Watt, dtype=np.float32)
    gw_a = np.ascontiguousarray(gw, dtype=np.float32)

    att_pad_full = np.zeros((B, H, TP), dtype=np.float32)
    att_pad_full[:, :, FILT:FILT + T] = att_prev
    maskneg = np.where(
        np.arange(T)[None, :] < lens[:, None], 0.0, -1e30
    ).astype(ml_dtypes.bfloat16)                           # (B, T)
    masknb_full = np.broadcast_to(
        maskneg[:, None, :], (B, H, T)
    )                                                      # (B, H, T)

    in_maps = []
    for i in range(NCORES):
        sl = slice(i * BL, (i + 1) * BL)
        qzc = np.zeros((DQ, BL), dtype=np.float32)
        qzc[:D, :] = dec_z[sl].T
        qzc[D, :] = 1.0
        in_maps.append(dict(
            enc=enc_hs_pad[sl],
            att_pad=np.ascontiguousarray(att_pad_full[sl]),
            masknb=np.ascontiguousarray(masknb_full[sl]),
            qz=qzc,
            wq=wq_aug, wk=wk_a, watt=watt_a, cwt=cwt, gw=gw_a,
            wv=wv_a, wo=wo_a, ident=ident,
        ))
    return in_maps


def kernel(**inputs):
    nc = get_nc()
    in_maps = make_in_maps(**inputs)
    res = run_bass_kernel_spmd(nc, in_maps, core_ids=list(range(NCORES)))
    c = np.concatenate([res.results[i]["c_out"] for i in range(NCORES)], axis=0)
    w = np.concatenate([res.results[i]["w_out"] for i in range(NCORES)], axis=0)
    return c, w


# revision 5
# speedup vs baseline: 1.0702x; 1.0702x over previous
"""Trainium2 Bass kernel for multi-head location-aware attention.

Full-input contract: kernel(**inputs) takes the unsharded numpy inputs and
returns (c, w) like the reference. Internally shards batch B=32 across 8
NeuronCores (4 batches/core, all 4 heads per core).

Math notes:
  - v-projection is algebraically eliminated: c = ((w @ enc) @ Wv) @ Wo.
  - gb cancels in softmax (shift invariance) and is dropped.
  - q-projection bias bq is folded via an augmented row (ones) in dec_z^T.
  - masking is host-precomputed as additive -1e30 rows (bf16).
  - matmul operands in bf16 (accumulation fp32 in PSUM); scores/softmax fp32.
"""

import numpy as np
import ml_dtypes
from contextlib import ExitStack

import concourse.bass as bass
import concourse.tile as tile
from concourse import bacc, mybir
from concourse.bass_utils import run_bass_kernel_spmd

FP32 = mybir.dt.float32
BF16 = mybir.dt.bfloat16
AF = mybir.ActivationFunctionType
AX = mybir.AxisListType
OP = mybir.AluOpType

# problem dims (hardcoded)
B, T, E, D, H, DK, DV, C, FILT = 32, 2048, 1024, 1024, 4, 512, 512, 10, 100
KW = 2 * FILT + 1            # 201
NCORES = 8
BL = B // NCORES             # 4 batches per core
TP = T + 2 * FILT            # 2248 padded att length
SCALING = 2.0
DQ = 1152                    # 1024 + 1 (ones row) padded to 9*128

NTT = T // 128               # 16 t-tiles
NEE = E // 128               # 8 e-tiles
NMK = DK // 128              # 4 dk-tiles
NCH = T // 512               # 4 t-chunks of 512
NKQ = DQ // 128              # 9 q contraction tiles

_CACHED_NC = None


def build_nc():
    nc = bacc.Bacc("TRN2", target_bir_lowering=False, debug=False)

    enc = nc.dram_tensor("enc", [BL, T, E], FP32, kind="ExternalInput").ap()
    att_pad = nc.dram_tensor("att_pad", [BL, H, TP], BF16, kind="ExternalInput").ap()
    masknb = nc.dram_tensor("masknb", [BL, H, T], BF16, kind="ExternalInput").ap()
    qz = nc.dram_tensor("qz", [DQ, BL], BF16, kind="ExternalInput").ap()
    wq = nc.dram_tensor("wq", [H, DQ, DK], BF16, kind="ExternalInput").ap()
    wk = nc.dram_tensor("wk", [H, E, DK], BF16, kind="ExternalInput").ap()
    watt = nc.dram_tensor("watt", [H, C, DK], BF16, kind="ExternalInput").ap()
    cwt = nc.dram_tensor("cwt", [H, KW, C], BF16, kind="ExternalInput").ap()
    gw = nc.dram_tensor("gw", [H, DK], BF16, kind="ExternalInput").ap()
    wv = nc.dram_tensor("wv", [H, E, DV], BF16, kind="ExternalInput").ap()
    wo = nc.dram_tensor("wo", [H * DV, E], BF16, kind="ExternalInput").ap()
    ident = nc.dram_tensor("ident", [128, 128], BF16, kind="ExternalInput").ap()

    c_out = nc.dram_tensor("c_out", [BL, E], FP32, kind="ExternalOutput").ap()
    w_out = nc.dram_tensor("w_out", [BL, H, T], FP32, kind="ExternalOutput").ap()

    with tile.TileContext(nc) as tc, ExitStack() as ctx:
        # ---------------- persistent pools ----------------
        const_p = ctx.enter_context(tc.tile_pool(name="const", bufs=1))
        wk_p = ctx.enter_context(tc.tile_pool(name="wkp", bufs=1))
        persist_p = ctx.enter_context(tc.tile_pool(name="persistp", bufs=1))
        sm_p = ctx.enter_context(tc.tile_pool(name="smp", bufs=1))

        # PSUM pools (8 banks total: 2+2+1+1+2)
        ps_mm = ctx.enter_context(tc.tile_pool(name="psmm", bufs=2, space="PSUM"))
        ps_tp = ctx.enter_context(tc.tile_pool(name="pstp", bufs=2, space="PSUM"))
        ps_e = ctx.enter_context(tc.tile_pool(name="pse", bufs=1, space="PSUM"))
        ps_cv = ctx.enter_context(tc.tile_pool(name="pscv", bufs=1, space="PSUM"))
        ps_misc = ctx.enter_context(tc.tile_pool(name="psmisc", bufs=1, space="PSUM"))

        # ---------------- constants / weights ----------------
        id_sb = const_p.tile([128, 128], BF16, tag="id")
        nc.sync.dma_start(id_sb[:], ident)

        wk_sb = wk_p.tile([128, H, NEE, DK], BF16, tag="wk")
        for h in range(H):
            for ke in range(NEE):
                nc.sync.dma_start(
                    wk_sb[:, h, ke, :], wk[h, ke * 128:(ke + 1) * 128, :]
                )

        watt_sb = const_p.tile([C, H, DK], BF16, tag="watt")
        cw0_sb = const_p.tile([128, H, C], BF16, tag="cw0")
        cw1_sb = const_p.tile([KW - 128, H, C], BF16, tag="cw1")
        gw_sb = const_p.tile([128, H, NMK], BF16, tag="gw")
        for h in range(H):
            nc.sync.dma_start(watt_sb[:, h, :], watt[h])
            nc.sync.dma_start(cw0_sb[:, h, :], cwt[h, 0:128, :])
            nc.sync.dma_start(cw1_sb[:, h, :], cwt[h, 128:KW, :])
            nc.sync.dma_start(
                gw_sb[:, h, :], bass.AP(gw.tensor, h * DK, [[1, 128], [128, NMK]])
            )

        mask_sb = const_p.tile([H, BL, T], BF16, tag="mask")
        for b in range(BL):
            nc.sync.dma_start(mask_sb[:, b, :], masknb[b])

        # persistent cross-phase tiles
        q_sb = persist_p.tile([128, H, NMK, BL], FP32, tag="q")
        ctxT_sb = persist_p.tile([128, NEE, BL, H], BF16, tag="ctxT")
        wT_sb = persist_p.tile([128, BL, NTT, H], BF16, tag="wT")

        # ---------------- q projection (once) ----------------
        with tc.tile_pool(name="wqp", bufs=2) as wq_p:
            qz_sb = const_p.tile([128, NKQ, BL], BF16, tag="qz")
            for kq in range(NKQ):
                nc.sync.dma_start(qz_sb[:, kq, :], qz[kq * 128:(kq + 1) * 128, :])
            for h in range(H):
                for mk in range(NMK):
                    wq_t = wq_p.tile([128, NKQ, 128], BF16, tag="wqt")
                    for kq in range(NKQ):
                        nc.sync.dma_start(
                            wq_t[:, kq, :],
                            wq[h, kq * 128:(kq + 1) * 128,
                               mk * 128:(mk + 1) * 128],
                        )
                    qps = ps_misc.tile([128, BL], FP32, tag="misc")
                    for kq in range(NKQ):
                        nc.tensor.matmul(
                            qps[:], wq_t[:, kq, :], qz_sb[:, kq, :],
                            start=(kq == 0), stop=(kq == NKQ - 1),
                        )
                    nc.vector.tensor_copy(q_sb[:, h, mk, :], qps[:])

        # ---------------- main loop over local batches ----------------
        with tc.tile_pool(name="encp", bufs=3) as enc_p, \
             tc.tile_pool(name="encbp", bufs=1) as encb_p, \
             tc.tile_pool(name="encTp", bufs=2) as encT_p, \
             tc.tile_pool(name="imp", bufs=2) as im_p, \
             tc.tile_pool(name="convp", bufs=2) as conv_p, \
             tc.tile_pool(name="tanhp", bufs=6) as tanh_p, \
             tc.tile_pool(name="ebp", bufs=4) as eb_p:
            for b in range(BL):
                E_b = sm_p.tile([H, T], FP32, tag="Eb")
                # enc[b] cast to bf16, kept resident for ctx matmul
                enc_bf = encb_p.tile([128, NTT, E], BF16, tag="encbf")
                for tch in range(NCH):
                    t0 = tch * 512
                    # --- A: cast 4 t-tiles to bf16 and transpose into encT ---
                    encT = encT_p.tile([128, NEE, 512], BF16, tag="encT")
                    for ti in range(4):
                        tt = tch * 4 + ti
                        et = enc_p.tile([128, E], FP32, tag="encn")
                        nc.sync.dma_start(et[:], enc[b, tt * 128:(tt + 1) * 128, :])
                        nc.vector.tensor_copy(enc_bf[:, tt, :], et[:])
                        for eg in range(2):
                            tps = ps_tp.tile([128, 512], BF16, tag="tp")
                            for j in range(4):
                                ee = eg * 4 + j
                                nc.tensor.transpose(
                                    tps[:, j * 128:(j + 1) * 128],
                                    enc_bf[:, tt, ee * 128:(ee + 1) * 128],
                                    id_sb[:],
                                )
                            nc.vector.tensor_copy(
                                encT[:, eg * 4:(eg + 1) * 4,
                                     ti * 128:(ti + 1) * 128],
                                tps[:].rearrange("p (e t) -> p e t", e=4),
                            )

                    # --- B: scores for this chunk, all heads ---
                    for h in range(H):
                        # conv: (C, 512)
                        im0 = im_p.tile([128, 512], BF16, tag="im0")
                        im1 = im_p.tile([KW - 128, 512], BF16, tag="im1")
                        base = (b * H + h) * TP
                        nc.sync.dma_start(
                            im0[:],
                            bass.AP(att_pad.tensor, base + t0,
                                    [[1, 128], [1, 512]]),
                        )
                        nc.sync.dma_start(
                            im1[:],
                            bass.AP(att_pad.tensor, base + t0 + 128,
                                    [[1, KW - 128], [1, 512]]),
                        )
                        cps = ps_cv.tile([C, 512], FP32, tag="cv")
                        nc.tensor.matmul(cps[:], cw0_sb[:, h, :], im0[:],
                                         start=True, stop=False)
                        nc.tensor.matmul(cps[:], cw1_sb[:, h, :], im1[:],
                                         start=False, stop=True)
                        conv_sb = conv_p.tile([C, 512], BF16, tag="conv")
                        nc.vector.tensor_copy(conv_sb[:], cps[:])

                        # k-proj + loc per dk tile, tanh
                        tanh_tiles = []
                        for mk in range(NMK):
                            kps = ps_mm.tile([128, 512], FP32, tag="kps")
                            for ke in range(NEE):
                                nc.tensor.matmul(
                                    kps[:],
                                    wk_sb[:, h, ke, mk * 128:(mk + 1) * 128],
                                    encT[:, ke, :],
                                    start=(ke == 0), stop=False,
                                )
                            nc.tensor.matmul(
                                kps[:],
                                watt_sb[:, h, mk * 128:(mk + 1) * 128],
                                conv_sb[:],
                                start=False, stop=True,
                            )
                            th = tanh_p.tile([128, 512], BF16, tag="tanh")
                            nc.scalar.activation(
                                th[:], kps[:], AF.Tanh,
                                bias=q_sb[:, h, mk, b:b + 1], scale=1.0,
                            )
                            tanh_tiles.append(th)

                        # gw dot -> e chunk (1, 512) -> E_b row h
                        eps = ps_e.tile([1, 512], FP32, tag="eps")
                        for mk in range(NMK):
                            nc.tensor.matmul(
                                eps[:], gw_sb[:, h, mk:mk + 1],
                                tanh_tiles[mk][:],
                                start=(mk == 0), stop=(mk == NMK - 1),
                            )
                        esb = eb_p.tile([1, 512], FP32, tag="esb")
                        nc.scalar.copy(esb[:], eps[:])
                        nc.sync.dma_start(E_b[h:h + 1, t0:t0 + 512], esb[:])

                # --- C: softmax over T for 4 heads (in place on E_b) ---
                nc.vector.tensor_tensor(E_b[:], E_b[:], mask_sb[:, b, :], op=OP.add)
                mx = sm_p.tile([H, 1], FP32, tag="mx")
                nc.vector.tensor_reduce(mx[:], E_b[:], AX.X, OP.max)
                m2 = sm_p.tile([H, 1], FP32, tag="m2")
                nc.vector.tensor_scalar_mul(m2[:], mx[:], -SCALING)
                S_b = sm_p.tile([H, 1], FP32, tag="Sb")
                nc.scalar.activation(E_b[:], E_b[:], AF.Exp, bias=m2[:],
                                     scale=SCALING, accum_out=S_b[:])
                sinv = sm_p.tile([H, 1], FP32, tag="sinv")
                nc.vector.reciprocal(sinv[:], S_b[:])
                nc.vector.tensor_scalar_mul(E_b[:], E_b[:], sinv[:])
                nc.sync.dma_start(w_out[b], E_b[:])

                # --- wT: transpose W rows to (t, h) bf16 ---
                Wbf = sm_p.tile([H, T], BF16, tag="Wbf")
                nc.vector.tensor_copy(Wbf[:], E_b[:])
                for g in range(NTT // 4):
                    tps = ps_tp.tile([128, 512], BF16, tag="tp")
                    for j in range(4):
                        tt = g * 4 + j
                        nc.tensor.transpose(
                            tps[:, j * 128:j * 128 + H],
                            Wbf[:, tt * 128:(tt + 1) * 128],
                            id_sb[0:H, 0:H],
                        )
                    for j in range(4):
                        tt = g * 4 + j
                        nc.vector.tensor_copy(
                            wT_sb[:, b, tt, :], tps[:, j * 128:j * 128 + H]
                        )

                # --- D: ctx = w @ enc_bf -> transpose -> ctxT (bf16) ---
                ctxps = ps_misc.tile([H, E], FP32, tag="misc")
                for tt in range(NTT):
                    for nch in range(2):
                        nc.tensor.matmul(
                            ctxps[:, nch * 512:(nch + 1) * 512],
                            wT_sb[:, b, tt, :],
                            enc_bf[:, tt, nch * 512:(nch + 1) * 512],
                            start=(tt == 0), stop=(tt == NTT - 1),
                        )
                ctxN = sm_p.tile([H, E], BF16, tag="ctxN")
                nc.vector.tensor_copy(ctxN[:], ctxps[:])
                for eg in range(2):
                    tps = ps_tp.tile([128, 512], BF16, tag="tp")
                    for j in range(4):
                        ee = eg * 4 + j
                        nc.tensor.transpose(
                            tps[:, j * 128:j * 128 + H],
                            ctxN[:, ee * 128:(ee + 1) * 128],
                            id_sb[0:H, 0:H],
                        )
                    for j in range(4):
                        ee = eg * 4 + j
                        nc.vector.tensor_copy(
                            ctxT_sb[:, ee, b, :], tps[:, j * 128:j * 128 + H]
                        )

        # ---------------- final: ctxv = ctxT.T @ Wv ; c = ctxv.T @ Wo ----
        with tc.tile_pool(name="wvp", bufs=2) as wv_p, \
             tc.tile_pool(name="wop", bufs=2) as wo_p:
            cvT_sb = persist_p.tile([128, H, NMK, BL], BF16, tag="cvT")
            for h in range(H):
                cvps = ps_cv.tile([BL, DV], FP32, tag="cv")
                for ee in range(NEE):
                    wv_t = wv_p.tile([128, DV], BF16, tag="wvt")
                    nc.sync.dma_start(
                        wv_t[:], wv[h, ee * 128:(ee + 1) * 128, :]
                    )
                    nc.tensor.matmul(
                        cvps[:], ctxT_sb[:, ee, :, h], wv_t[:],
                        start=(ee == 0), stop=(ee == NEE - 1),
                    )
                cv_sb = sm_p.tile([BL, DV], BF16, tag="cv")
                nc.vector.tensor_copy(cv_sb[:], cvps[:])
                for mv in range(NMK):
                    tps = ps_tp.tile([128, 512], BF16, tag="tp")
                    nc.tensor.transpose(
                        tps[:, 0:BL], cv_sb[:, mv * 128:(mv + 1) * 128],
                        id_sb[0:BL, 0:BL],
                    )
                    nc.vector.tensor_copy(cvT_sb[:, h, mv, :], tps[:, 0:BL])

            cps_f = ps_misc.tile([BL, E], FP32, tag="misc")
            nkt = H * NMK
            k = 0
            for h in range(H):
                for mv in range(NMK):
                    wo_t = wo_p.tile([128, E], BF16, tag="wot")
                    nc.sync.dma_start(
                        wo_t[:],
                        wo[(h * DV + mv * 128):(h * DV + (mv + 1) * 128), :],
                    )
                    for nch in range(2):
                        nc.tensor.matmul(
                            cps_f[:, nch * 512:(nch + 1) * 512],
                            cvT_sb[:, h, mv, :],
                            wo_t[:, nch * 512:(nch + 1) * 512],
                            start=(k == 0), stop=(k == nkt - 1),
                        )
                    k += 1
            c_sb = sm_p.tile([BL, E], FP32, tag="csb")
            nc.vector.tensor_copy(c_sb[:], cps_f[:])
            nc.sync.dma_start(c_out, c_sb[:])

    nc.compile()
    return nc


def get_nc():
    global _CACHED_NC
    if _CACHED_NC is None:
        _CACHED_NC = build_nc()
    return _CACHED_NC


def make_in_maps(enc_hs_pad, enc_hs_len, dec_z, att_prev, Wq, bq, Wk, Wv, gw,
                 gb, conv_w, Watt, Wo):
    bf = ml_dtypes.bfloat16
    enc_hs_pad = np.ascontiguousarray(enc_hs_pad, dtype=np.float32)
    dec_z = np.asarray(dec_z, dtype=np.float32)
    att_prev = np.asarray(att_prev, dtype=np.float32)
    lens = np.asarray(enc_hs_len)

    wq_aug = np.zeros((H, DQ, DK), dtype=np.float32)
    wq_aug[:, :D, :] = Wq
    wq_aug[:, D, :] = bq
    wq_aug = wq_aug.astype(bf)
    cwt = np.ascontiguousarray(np.transpose(conv_w, (0, 2, 1))).astype(bf)
    ident = np.eye(128, dtype=np.float32).astype(bf)
    wk_a = np.ascontiguousarray(Wk).astype(bf)
    wv_a = np.ascontiguousarray(Wv).astype(bf)
    wo_a = np.ascontiguousarray(Wo).astype(bf)
    watt_a = np.ascontiguousarray(Watt).astype(bf)
    gw_a = np.ascontiguousarray(gw).astype(bf)

    att_pad_full = np.zeros((B, H, TP), dtype=np.float32)
    att_pad_full[:, :, FILT:FILT + T] = att_prev
    att_pad_full = att_pad_full.astype(bf)
    maskneg = np.where(
        np.arange(T)[None, :] < lens[:, None], 0.0, -1e30
    ).astype(bf)                                           # (B, T)
    masknb_full = np.broadcast_to(maskneg[:, None, :], (B, H, T))

    in_maps = []
    for i in range(NCORES):
        sl = slice(i * BL, (i + 1) * BL)
        qzc = np.zeros((DQ, BL), dtype=np.float32)
        qzc[:D, :] = dec_z[sl].T
        qzc[D, :] = 1.0
        in_maps.append(dict(
            enc=enc_hs_pad[sl],
            att_pad=np.ascontiguousarray(att_pad_full[sl]),
            masknb=np.ascontiguousarray(masknb_full[sl]),
            qz=qzc.astype(bf),
            wq=wq_aug, wk=wk_a, watt=watt_a, cwt=cwt, gw=gw_a,
            wv=wv_a, wo=wo_a, ident=ident,
        ))
    return in_maps


def kernel(**inputs):
    nc = get_nc()
    in_maps = make_in_maps(**inputs)
    res = run_bass_kernel_spmd(nc, in_maps, core_ids=list(range(NCORES)))
    c = np.concatenate([res.results[i]["c_out"] for i in range(NCORES)], axis=0)
    w = np.concatenate([res.results[i]["w_out"] for i in range(NCORES)], axis=0)
    return c, w


# revision 6
# speedup vs baseline: 3.1929x; 2.9835x over previous
"""Trainium2 Bass kernel for multi-head location-aware attention.

Full-input contract: kernel(**inputs) takes the unsharded numpy inputs and
returns (c, w) like the reference. Internally shards batch B=32 across 8
NeuronCores (4 batches/core, all 4 heads per core).

Math notes:
  - v-projection is algebraically eliminated: c = ((w @ enc) @ Wv) @ Wo.
  - gb cancels in softmax (shift invariance) and is dropped.
  - q-projection bias bq is folded via an augmented row (ones) in dec_z^T.
  - masking is host-precomputed as additive -1e30 rows (bf16).
  - matmul operands in bf16 (accumulation fp32 in PSUM); scores/softmax fp32.
"""

import numpy as np
import ml_dtypes
from contextlib import ExitStack

import concourse.bass as bass
import concourse.tile as tile
from concourse import bacc, mybir
from concourse.bass_utils import run_bass_kernel_spmd

FP32 = mybir.dt.float32
BF16 = mybir.dt.bfloat16
AF = mybir.ActivationFunctionType
AX = mybir.AxisListType
OP = mybir.AluOpType

# problem dims (hardcoded)
B, T, E, D, H, DK, DV, C, FILT = 32, 2048, 1024, 1024, 4, 512, 512, 10, 100
KW = 2 * FILT + 1            # 201
NCORES = 8
BL = B // NCORES             # 4 batches per core
TP = T + 2 * FILT            # 2248 padded att length
SCALING = 2.0
DQ = 1152                    # 1024 + 1 (ones row) padded to 9*128

NTT = T // 128               # 16 t-tiles
NEE = E // 128               # 8 e-tiles
NMK = DK // 128              # 4 dk-tiles
NCH = T // 512               # 4 t-chunks of 512
NKQ = DQ // 128              # 9 q contraction tiles

_CACHED_NC = None


def build_nc():
    nc = bacc.Bacc("TRN2", target_bir_lowering=False, debug=False)

    enc = nc.dram_tensor("enc", [BL, T, E], FP32, kind="ExternalInput").ap()
    att_pad = nc.dram_tensor("att_pad", [BL, H, TP], BF16, kind="ExternalInput").ap()
    masknb = nc.dram_tensor("masknb", [BL, H, T], BF16, kind="ExternalInput").ap()
    qz = nc.dram_tensor("qz", [DQ, BL], BF16, kind="ExternalInput").ap()
    wq = nc.dram_tensor("wq", [H, DQ, DK], BF16, kind="ExternalInput").ap()
    wk = nc.dram_tensor("wk", [H, E, DK], BF16, kind="ExternalInput").ap()
    watt = nc.dram_tensor("watt", [H, C, DK], BF16, kind="ExternalInput").ap()
    cwt = nc.dram_tensor("cwt", [H, KW, C], BF16, kind="ExternalInput").ap()
    gw = nc.dram_tensor("gw", [H, DK], BF16, kind="ExternalInput").ap()
    wv = nc.dram_tensor("wv", [H, E, DV], BF16, kind="ExternalInput").ap()
    wo = nc.dram_tensor("wo", [H * DV, E], BF16, kind="ExternalInput").ap()
    ident = nc.dram_tensor("ident", [128, 128], BF16, kind="ExternalInput").ap()

    c_out = nc.dram_tensor("c_out", [BL, E], FP32, kind="ExternalOutput").ap()
    w_out = nc.dram_tensor("w_out", [BL, H, T], FP32, kind="ExternalOutput").ap()

    with tile.TileContext(nc) as tc, ExitStack() as ctx:
        # ---------------- persistent pools ----------------
        const_p = ctx.enter_context(tc.tile_pool(name="const", bufs=1))
        wk_p = ctx.enter_context(tc.tile_pool(name="wkp", bufs=1))
        persist_p = ctx.enter_context(tc.tile_pool(name="persistp", bufs=1))
        sm_p = ctx.enter_context(tc.tile_pool(name="smp", bufs=1))

        # PSUM pools (8 banks total: 2+2+1+1+2)
        ps_mm = ctx.enter_context(tc.tile_pool(name="psmm", bufs=3, space="PSUM"))
        ps_tp = ctx.enter_context(tc.tile_pool(name="pstp", bufs=2, space="PSUM"))
        ps_e = ctx.enter_context(tc.tile_pool(name="pse", bufs=1, space="PSUM"))
        ps_cv = ctx.enter_context(tc.tile_pool(name="pscv", bufs=1, space="PSUM"))

        # ---------------- constants / weights ----------------
        id_sb = const_p.tile([128, 128], BF16, tag="id")
        nc.sync.dma_start(id_sb[:], ident)

        wk_sb = wk_p.tile([128, H, NEE, DK], BF16, tag="wk")
        for h in range(H):
            for ke in range(NEE):
                nc.sync.dma_start(
                    wk_sb[:, h, ke, :], wk[h, ke * 128:(ke + 1) * 128, :]
                )

        watt_sb = const_p.tile([C, H, DK], BF16, tag="watt")
        cw0_sb = const_p.tile([128, H, C], BF16, tag="cw0")
        cw1_sb = const_p.tile([KW - 128, H, C], BF16, tag="cw1")
        gw_sb = const_p.tile([128, H, NMK], BF16, tag="gw")
        for h in range(H):
            nc.sync.dma_start(watt_sb[:, h, :], watt[h])
            nc.sync.dma_start(cw0_sb[:, h, :], cwt[h, 0:128, :])
            nc.sync.dma_start(cw1_sb[:, h, :], cwt[h, 128:KW, :])
            nc.sync.dma_start(
                gw_sb[:, h, :], bass.AP(gw.tensor, h * DK, [[1, 128], [128, NMK]])
            )

        mask_sb = const_p.tile([H, BL, T], BF16, tag="mask")
        for b in range(BL):
            nc.sync.dma_start(mask_sb[:, b, :], masknb[b])

        # persistent cross-phase tiles
        q_sb = persist_p.tile([128, H, NMK, BL], FP32, tag="q")
        ctxT_sb = persist_p.tile([128, NEE, BL, H], BF16, tag="ctxT")
        wT_sb = persist_p.tile([128, BL, NTT, H], BF16, tag="wT")

        # ---------------- q projection (once) ----------------
        with tc.tile_pool(name="wqp", bufs=2) as wq_p:
            qz_sb = const_p.tile([128, NKQ, BL], BF16, tag="qz")
            for kq in range(NKQ):
                nc.sync.dma_start(qz_sb[:, kq, :], qz[kq * 128:(kq + 1) * 128, :])
            for h in range(H):
                for mk in range(NMK):
                    wq_t = wq_p.tile([128, NKQ, 128], BF16, tag="wqt")
                    for kq in range(NKQ):
                        nc.sync.dma_start(
                            wq_t[:, kq, :],
                            wq[h, kq * 128:(kq + 1) * 128,
                               mk * 128:(mk + 1) * 128],
                        )
                    qps = ps_mm.tile([128, BL], FP32, tag="kps")
                    for kq in range(NKQ):
                        nc.tensor.matmul(
                            qps[:], wq_t[:, kq, :], qz_sb[:, kq, :],
                            start=(kq == 0), stop=(kq == NKQ - 1),
                        )
                    nc.vector.tensor_copy(q_sb[:, h, mk, :], qps[:])

        # ---------------- main loop over local batches ----------------
        with tc.tile_pool(name="encbp", bufs=1) as encb_p, \
             tc.tile_pool(name="encTp", bufs=2) as encT_p, \
             tc.tile_pool(name="imp", bufs=2) as im_p, \
             tc.tile_pool(name="convp", bufs=2) as conv_p, \
             tc.tile_pool(name="tanhp", bufs=6) as tanh_p, \
             tc.tile_pool(name="ebp", bufs=5) as eb_p:
            for b in range(BL):
                E_b = sm_p.tile([H, T], FP32, tag="Eb")
                e_rows = {}
                for h in range(H):
                    e_rows[h] = eb_p.tile([1, T], FP32, tag="esb",
                                          name=f"esb{h}")
                # enc[b] cast to bf16, kept resident for ctx matmul
                enc_bf = encb_p.tile([128, NTT, E], BF16, tag="encbf")
                for tch in range(NCH):
                    t0 = tch * 512
                    # --- A: cast 4 t-tiles to bf16 and transpose into encT ---
                    encT = encT_p.tile([128, NEE, 512], BF16, tag="encT")
                    src = bass.AP(enc.tensor, b * T * E + tch * 4 * 128 * E,
                                  [[E, 128], [128 * E, 4], [1, E]])
                    nc.gpsimd.dma_start(enc_bf[:, tch * 4:(tch + 1) * 4, :], src)
                    for ti in range(4):
                        tt = tch * 4 + ti
                        for eg in range(2):
                            tps = ps_tp.tile([128, 512], BF16, tag="tp")
                            for j in range(4):
                                ee = eg * 4 + j
                                nc.tensor.transpose(
                                    tps[:, j * 128:(j + 1) * 128],
                                    enc_bf[:, tt, ee * 128:(ee + 1) * 128],
                                    id_sb[:],
                                )
                            nc.vector.tensor_copy(
                                encT[:, eg * 4:(eg + 1) * 4,
                                     ti * 128:(ti + 1) * 128],
                                tps[:].rearrange("p (e t) -> p e t", e=4),
                            )

                    # --- B: scores for this chunk, all heads ---
                    for h in range(H):
                        # conv: (C, 512)
                        im0 = im_p.tile([128, 512], BF16, tag="im0")
                        im1 = im_p.tile([KW - 128, 512], BF16, tag="im1")
                        base = (b * H + h) * TP
                        nc.sync.dma_start(
                            im0[:],
                            bass.AP(att_pad.tensor, base + t0,
                                    [[1, 128], [1, 512]]),
                        )
                        nc.sync.dma_start(
                            im1[:],
                            bass.AP(att_pad.tensor, base + t0 + 128,
                                    [[1, KW - 128], [1, 512]]),
                        )
                        cps = ps_cv.tile([C, 512], FP32, tag="cv")
                        nc.tensor.matmul(cps[:], cw0_sb[:, h, :], im0[:],
                                         start=True, stop=False)
                        nc.tensor.matmul(cps[:], cw1_sb[:, h, :], im1[:],
                                         start=False, stop=True)
                        conv_sb = conv_p.tile([C, 512], BF16, tag="conv")
                        nc.vector.tensor_copy(conv_sb[:], cps[:])

                        # k-proj + loc per dk tile, tanh
                        tanh_tiles = []
                        for mk in range(NMK):
                            kps = ps_mm.tile([128, 512], FP32, tag="kps")
                            for ke in range(NEE):
                                nc.tensor.matmul(
                                    kps[:],
                                    wk_sb[:, h, ke, mk * 128:(mk + 1) * 128],
                                    encT[:, ke, :],
                                    start=(ke == 0), stop=False,
                                )
                            nc.tensor.matmul(
                                kps[:],
                                watt_sb[:, h, mk * 128:(mk + 1) * 128],
                                conv_sb[:],
                                start=False, stop=True,
                            )
                            th = tanh_p.tile([128, 512], BF16, tag="tanh")
                            nc.scalar.activation(
                                th[:], kps[:], AF.Tanh,
                                bias=q_sb[:, h, mk, b:b + 1], scale=1.0,
                            )
                            tanh_tiles.append(th)

                        # gw dot -> e chunk (1, 512) -> E_b row h
                        eps = ps_e.tile([1, 512], FP32, tag="eps")
                        for mk in range(NMK):
                            nc.tensor.matmul(
                                eps[:], gw_sb[:, h, mk:mk + 1],
                                tanh_tiles[mk][:],
                                start=(mk == 0), stop=(mk == NMK - 1),
                            )
                        nc.scalar.copy(e_rows[h][:, t0:t0 + 512], eps[:])

                # --- C: softmax over T for 4 heads (in place on E_b) ---
                for h in range(H):
                    nc.sync.dma_start(E_b[h:h + 1, :], e_rows[h][:])
                nc.vector.tensor_tensor(E_b[:], E_b[:], mask_sb[:, b, :], op=OP.add)
                mx = sm_p.tile([H, 1], FP32, tag="mx")
                nc.vector.tensor_reduce(mx[:], E_b[:], AX.X, OP.max)
                m2 = sm_p.tile([H, 1], FP32, tag="m2")
                nc.vector.tensor_scalar_mul(m2[:], mx[:], -SCALING)
                S_b = sm_p.tile([H, 1], FP32, tag="Sb")
                nc.scalar.activation(E_b[:], E_b[:], AF.Exp, bias=m2[:],
                                     scale=SCALING, accum_out=S_b[:])
                sinv = sm_p.tile([H, 1], FP32, tag="sinv")
                nc.vector.reciprocal(sinv[:], S_b[:])
                nc.vector.tensor_scalar_mul(E_b[:], E_b[:], sinv[:])
                nc.sync.dma_start(w_out[b], E_b[:])

                # --- wT: transpose W rows to (t, h) bf16 ---
                Wbf = sm_p.tile([H, T], BF16, tag="Wbf")
                nc.vector.tensor_copy(Wbf[:], E_b[:])
                for g in range(NTT // 4):
                    tps = ps_tp.tile([128, 512], BF16, tag="tp")
                    for j in range(4):
                        tt = g * 4 + j
                        nc.tensor.transpose(
                            tps[:, j * 128:j * 128 + H],
                            Wbf[:, tt * 128:(tt + 1) * 128],
                            id_sb[0:H, 0:H],
                        )
                    for j in range(4):
                        tt = g * 4 + j
                        nc.vector.tensor_copy(
                            wT_sb[:, b, tt, :], tps[:, j * 128:j * 128 + H]
                        )

                # --- D: ctx = w @ enc_bf -> transpose -> ctxT (bf16) ---
                ctxN = sm_p.tile([H, E], BF16, tag="ctxN")
                for nch in range(2):
                    ctxps = ps_cv.tile([H, 512], FP32, tag="cv")
                    for tt in range(NTT):
                        nc.tensor.matmul(
                            ctxps[:],
                            wT_sb[:, b, tt, :],
                            enc_bf[:, tt, nch * 512:(nch + 1) * 512],
                            start=(tt == 0), stop=(tt == NTT - 1),
                        )
                    nc.vector.tensor_copy(ctxN[:, nch * 512:(nch + 1) * 512],
                                          ctxps[:])
                for eg in range(2):
                    tps = ps_tp.tile([128, 512], BF16, tag="tp")
                    for j in range(4):
                        ee = eg * 4 + j
                        nc.tensor.transpose(
                            tps[:, j * 128:j * 128 + H],
                            ctxN[:, ee * 128:(ee + 1) * 128],
                            id_sb[0:H, 0:H],
                        )
                    for j in range(4):
                        ee = eg * 4 + j
                        nc.vector.tensor_copy(
                            ctxT_sb[:, ee, b, :], tps[:, j * 128:j * 128 + H]
                        )

        # ---------------- final: ctxv = ctxT.T @ Wv ; c = ctxv.T @ Wo ----
        with tc.tile_pool(name="wvp", bufs=2) as wv_p, \
             tc.tile_pool(name="wop", bufs=2) as wo_p:
            cvT_sb = persist_p.tile([128, H, NMK, BL], BF16, tag="cvT")
            for h in range(H):
                cvps = ps_cv.tile([BL, DV], FP32, tag="cv")
                for ee in range(NEE):
                    wv_t = wv_p.tile([128, DV], BF16, tag="wvt")
                    nc.sync.dma_start(
                        wv_t[:], wv[h, ee * 128:(ee + 1) * 128, :]
                    )
                    nc.tensor.matmul(
                        cvps[:], ctxT_sb[:, ee, :, h], wv_t[:],
                        start=(ee == 0), stop=(ee == NEE - 1),
                    )
                cv_sb = sm_p.tile([BL, DV], BF16, tag="cv")
                nc.vector.tensor_copy(cv_sb[:], cvps[:])
                for mv in range(NMK):
                    tps = ps_tp.tile([128, 512], BF16, tag="tp")
                    nc.tensor.transpose(
                        tps[:, 0:BL], cv_sb[:, mv * 128:(mv + 1) * 128],
                        id_sb[0:BL, 0:BL],
                    )
                    nc.vector.tensor_copy(cvT_sb[:, h, mv, :], tps[:, 0:BL])

            nkt = H * NMK
            c_sb = sm_p.tile([BL, E], FP32, tag="csb")
            for nch in range(2):
                cps_f = ps_cv.tile([BL, 512], FP32, tag="cv")
                k = 0
                for h in range(H):
                    for mv in range(NMK):
                        wo_t = wo_p.tile([128, 512], BF16, tag="wot",
                                         name=f"wot{nch}_{h}_{mv}")
                        nc.sync.dma_start(
                            wo_t[:],
                            wo[(h * DV + mv * 128):(h * DV + (mv + 1) * 128),
                               nch * 512:(nch + 1) * 512],
                        )
                        nc.tensor.matmul(
                            cps_f[:],
                            cvT_sb[:, h, mv, :],
                            wo_t[:],
                            start=(k == 0), stop=(k == nkt - 1),
                        )
                        k += 1
                nc.vector.tensor_copy(c_sb[:, nch * 512:(nch + 1) * 512], cps_f[:])
            nc.sync.dma_start(c_out, c_sb[:])

    nc.compile()
    return nc


def get_nc():
    global _CACHED_NC
    if _CACHED_NC is None:
        _CACHED_NC = build_nc()
    return _CACHED_NC


def make_in_maps(enc_hs_pad, enc_hs_len, dec_z, att_prev, Wq, bq, Wk, Wv, gw,
                 gb, conv_w, Watt, Wo):
    bf = ml_dtypes.bfloat16
    enc_hs_pad = np.ascontiguousarray(enc_hs_pad, dtype=np.float32)
    dec_z = np.asarray(dec_z, dtype=np.float32)
    att_prev = np.asarray(att_prev, dtype=np.float32)
    lens = np.asarray(enc_hs_len)

    wq_aug = np.zeros((H, DQ, DK), dtype=np.float32)
    wq_aug[:, :D, :] = Wq
    wq_aug[:, D, :] = bq
    wq_aug = wq_aug.astype(bf)
    cwt = np.ascontiguousarray(np.transpose(conv_w, (0, 2, 1))).astype(bf)
    ident = np.eye(128, dtype=np.float32).astype(bf)
    wk_a = np.ascontiguousarray(Wk).astype(bf)
    wv_a = np.ascontiguousarray(Wv).astype(bf)
    wo_a = np.ascontiguousarray(Wo).astype(bf)
    watt_a = np.ascontiguousarray(Watt).astype(bf)
    gw_a = np.ascontiguousarray(gw).astype(bf)

    att_pad_full = np.zeros((B, H, TP), dtype=np.float32)
    att_pad_full[:, :, FILT:FILT + T] = att_prev
    att_pad_full = att_pad_full.astype(bf)
    maskneg = np.where(
        np.arange(T)[None, :] < lens[:, None], 0.0, -1e30
    ).astype(bf)                                           # (B, T)
    masknb_full = np.broadcast_to(maskneg[:, None, :], (B, H, T))

    in_maps = []
    for i in range(NCORES):
        sl = slice(i * BL, (i + 1) * BL)
        qzc = np.zeros((DQ, BL), dtype=np.float32)
        qzc[:D, :] = dec_z[sl].T
        qzc[D, :] = 1.0
        in_maps.append(dict(
            enc=enc_hs_pad[sl],
            att_pad=np.ascontiguousarray(att_pad_full[sl]),
            masknb=np.ascontiguousarray(masknb_full[sl]),
            qz=qzc.astype(bf),
            wq=wq_aug, wk=wk_a, watt=watt_a, cwt=cwt, gw=gw_a,
            wv=wv_a, wo=wo_a, ident=ident,
        ))
    return in_maps


def kernel(**inputs):
    nc = get_nc()
    in_maps = make_in_maps(**inputs)
    res = run_bass_kernel_spmd(nc, in_maps, core_ids=list(range(NCORES)))
    c = np.concatenate([res.results[i]["c_out"] for i in range(NCORES)], axis=0)
    w = np.concatenate([res.results[i]["w_out"] for i in range(NCORES)], axis=0)
    return c, w
